# revision 14
# baseline (speedup 1.0000x reference)
"""Trainium2 Bass kernel for nn_Memory (topk_masking).

Algorithm (per query row q of N=32768, item count 2048, K=10):
  logits l = q @ mempool.T
  e = exp(l); S = sum(e)                       (softmax, no max-shift: |l| <= ~3)
  top-10 threshold t10 via 8-way sectioned DVE max8 + small merge
  u = 1 + e/S  (~= exp(e/S) to 5e-6: e/S <= 4e-3)
  g = (e >= t10) * u;  Z = sum(g)
  out = (g @ mempool) / Z                      (masked dense matmul, no gather)

mm1 precision: fp16 main term (16*qh)@(16*mh) + one fp8-e4m3 DoubleRow matmul
carrying both correction terms (qh/32 paired with 2^13*ml, 2^8*ql paired with
mh) — every product lands at 2^8 scale in PSUM; exp folds in the 2^-8.
Logit noise sigma ~ 6.6e-6 -> ~6 selection flips per 32768 rows, rel ~5e-3.
DoubleRow runs at 0.5 cyc/row: mm1 = 12288 PE-cyc/tile vs 24576 for the
3-term fp16 split. mm2 dense fp16 (8192 cyc).

Schedule: mm2 skewed 2 tiles behind mm1 so the topk/mask/transpose chain of
tile t hides under mm1(t+1), mm1(t+2). DMAs spread over three queues to kill
head-of-line serialization: SP = loads, Act-HWDGE = g transposes,
Pool-SWDGE = out stores. Per-tile q data packed into a single 2KB/partition
transfer.

Sharding: data-parallel over queries. 32 units of [512 dim x 1024 queries]
(16 batches x 2 inputs); each of 8 cores takes 4 units = 32 tiles of 128
queries. mempool (4MB) replicated per core. Host does layout marshalling only.
"""
import sys
sys.path.insert(0, '/opt/trn_rl_repo')

import numpy as np
import ml_dtypes
import concourse.bacc as bacc
import concourse.mybir as mybir
import concourse.tile as tile
from concourse.bass_utils import run_bass_kernel_spmd

F32 = mybir.dt.float32
F16 = mybir.dt.float16
F8E4 = mybir.dt.float8e4
E4 = ml_dtypes.float8_e4m3

DIM = 512
NITEM = 2048
NCORES = 8
UNITS_PER_CORE = 4
QPU = 1024
TILES = UNITS_PER_CORE * QPU // 128
NEG = -1e30
EXP = mybir.ActivationFunctionType.Exp
COPY = mybir.ActivationFunctionType.Copy
DR = mybir.MatmulPerfMode.DoubleRow

_prog_cache = {}


def declare_io(nc):
    return {
        # per-tile packed q payload: [u, tile, partition, 1024 f16] where the
        # 2KB per partition = 1KB fp16 main operand + 1KB fp8 DR pair operand
        "qall": nc.declare_dram_parameter("qall", [UNITS_PER_CORE, 8, 128, 1024],
                                          F16, isOutput=False),
        "mh16": nc.declare_dram_parameter("mh16", [DIM, NITEM], F16, isOutput=False),
        "m8": nc.declare_dram_parameter("m8", [2, DIM, NITEM], F8E4, isOutput=False),
        "mp": nc.declare_dram_parameter("mp", [NITEM, DIM], F16, isOutput=False),
        "out": nc.declare_dram_parameter("out", [UNITS_PER_CORE * QPU, DIM], F32,
                                         isOutput=True),
    }


def emit(nc, tc, dram):
    with (
        tc.tile_pool(name="const", bufs=1) as cpool,
        tc.tile_pool(name="qin", bufs=3) as qpool,
        tc.tile_pool(name="work", bufs=2) as wpool,
        tc.tile_pool(name="carry3", bufs=3) as kpool,
        tc.tile_pool(name="outp", bufs=3) as opool,
        tc.tile_pool(name="ps_l", bufs=2, space="PSUM") as ps_l,
        tc.tile_pool(name="ps_o", bufs=2, space="PSUM") as ps_o,
    ):
        # consts as per-half tiles: dependency tracking is tile-granular, so
        # half-tiles let the first matmul group start as soon as its own half
        # has landed (three DMA queues deliver in first-needed order)
        mh_h = [cpool.tile([128, 4, 1024], F16, name=f"mh_h{i}") for i in range(2)]
        m8_h = [cpool.tile([128, 2, 4, 1024], F8E4, name=f"m8_h{i}") for i in range(2)]
        mp_sb = cpool.tile([128, 16, DIM], F16)

        qall0_sb = qpool.tile([128, 1024], F16, tag="qall", name="qall_sb")
        nc.sync.dma_start(qall0_sb[:], dram["qall"][0, 0])
        nc.sync.dma_start(mh_h[0][:],
                          dram["mh16"][:, 0:1024].rearrange("(kc p) n -> p kc n", p=128))
        nc.scalar.dma_start(m8_h[0][:],
                            dram["m8"][:, :, 0:1024].rearrange("two (kc p) n -> p two kc n", p=128))
        nc.scalar.dma_start(mh_h[1][:],
                            dram["mh16"][:, 1024:2048].rearrange("(kc p) n -> p kc n", p=128))
        nc.gpsimd.dma_start(m8_h[1][:],
                            dram["m8"][:, :, 1024:2048].rearrange("two (kc p) n -> p two kc n", p=128))
        nc.gpsimd.dma_start(mp_sb[:], dram["mp"][:].rearrange("(ic p) d -> p ic d", p=128))

        carry = {}

        def tile_mm1(t):
            u, tt = divmod(t, QPU // 128)
            if t == 0:
                qall_sb = qall0_sb
            else:
                qall_sb = qpool.tile([128, 1024], F16, tag="qall", name="qall_sb")
                nc.sync.dma_start(qall_sb[:], dram["qall"][u, tt])
            q16_sb = qall_sb[:, 0:512].rearrange("p (kc f) -> p kc f", kc=4)
            q8_sb = (qall_sb[:, 512:1024].bitcast(F8E4)
                     .rearrange("p (two kc f) -> p two kc f", two=2, kc=4))

            e_sb = wpool.tile([128, NITEM], F32, tag="e", name="e_sb")
            S_p = wpool.tile([128, 2], F32, tag="Sp", name="S_p")
            cand = wpool.tile([128, 64], F32, tag="cand", name="cand")
            for h in range(2):
                l_ps = ps_l.tile([128, 1024], F32, tag="l", name="l_ps")
                for c2 in range(2):
                    col = 512 * c2
                    dst = l_ps[:, 512 * c2:512 * (c2 + 1)]
                    for kc in range(4):
                        nc.tensor.matmul(dst, q16_sb[:, kc, :],
                                         mh_h[h][:, kc, col:col + 512],
                                         start=(kc == 0), stop=False)
                    for kc in range(4):
                        nc.tensor.matmul(dst, q8_sb[:, :, kc, :],
                                         m8_h[h][:, :, kc, col:col + 512],
                                         start=False, stop=(kc == 3), perf_mode=DR)
                nc.scalar.activation(e_sb[:, 1024 * h:1024 * (h + 1)], l_ps[:],
                                     EXP, scale=2.0**-8, accum_out=S_p[:, h:h + 1])
                # sectioned top-8 for this half overlaps the other half's matmul
                for s in range(4 * h, 4 * h + 4):
                    nc.vector.max(out=cand[:, 8 * s:8 * (s + 1)],
                                  in_=e_sb[:, 256 * s:256 * (s + 1)])

            S_sb = wpool.tile([128, 1], F32, tag="S", name="S_sb")
            nc.vector.tensor_add(S_sb[:], S_p[:, 0:1], S_p[:, 1:2])
            Sinv = wpool.tile([128, 1], F32, tag="Sinv", name="Sinv")
            nc.vector.reciprocal(Sinv[:], S_sb[:])

            top8 = wpool.tile([128, 8], F32, tag="top8", name="top8")
            candm = wpool.tile([128, 64], F32, tag="candm", name="candm")
            next8 = wpool.tile([128, 8], F32, tag="next8", name="next8")
            nc.vector.max(out=top8[:], in_=cand[:])
            nc.vector.match_replace(out=candm[:], in_to_replace=top8[:],
                                    in_values=cand[:], imm_value=NEG)
            nc.vector.max(out=next8[:], in_=candm[:])

            # g / transpose in item-halves: halves the drain latency of the
            # stt -> transpose -> mm2 chain (matters at the pipeline tail)
            u_sb = wpool.tile([128, NITEM], F16, tag="u", name="u_sb")
            g_sb = wpool.tile([128, NITEM], F16, tag="g", name="g_sb")
            Z_p = wpool.tile([128, 2], F32, tag="Zp", name="Z_p")
            gtA = kpool.tile([128, 8, 128], F16, tag="gtA", name="gtA")
            gtB = kpool.tile([128, 8, 128], F16, tag="gtB", name="gtB")
            for h, (gt_h, eng) in enumerate([(gtA, nc.scalar), (gtB, nc.scalar)]):
                cs = slice(1024 * h, 1024 * (h + 1))
                nc.scalar.activation(u_sb[:, cs], e_sb[:, cs], COPY,
                                     scale=Sinv[:], bias=1.0)
                nc.vector.scalar_tensor_tensor(
                    out=g_sb[:, cs], in0=e_sb[:, cs], scalar=next8[:, 1:2],
                    in1=u_sb[:, cs],
                    op0=mybir.AluOpType.is_ge, op1=mybir.AluOpType.mult,
                    accum_out=Z_p[:, h:h + 1])
                eng.dma_start_transpose(gt_h[:], g_sb[:, cs])
            Z_sb = wpool.tile([128, 1], F32, tag="Z", name="Z_sb")
            nc.vector.tensor_add(Z_sb[:], Z_p[:, 0:1], Z_p[:, 1:2])
            Zinv = kpool.tile([128, 1], F32, tag="Zinv", name="Zinv")
            nc.vector.reciprocal(Zinv[:], Z_sb[:])
            carry[t] = (gtA, gtB, Zinv)

        def tile_mm2(t):
            gtA, gtB, Zinv = carry.pop(t)
            o_ps = ps_o.tile([128, DIM], F32, tag="o", name="o_ps")
            for ic in range(16):
                gt_h = gtA if ic < 8 else gtB
                nc.tensor.matmul(o_ps[:], gt_h[:, ic % 8, :], mp_sb[:, ic, :],
                                 start=(ic == 0), stop=(ic == 15))
            o_sb = opool.tile([128, DIM], F32, tag="osb", name="o_sb")
            nc.scalar.activation(o_sb[:], o_ps[:], COPY, scale=Zinv[:])
            nc.gpsimd.dma_start(dram["out"][128 * t:128 * (t + 1), :], o_sb[:])

        # software pipeline, skew 2: the topk/mask/transpose chain of tile t
        # completes while PE runs mm1(t+1) and mm1(t+2)
        for t in range(TILES):
            tile_mm1(t)
            if t >= 2:
                tile_mm2(t - 2)
        tile_mm2(TILES - 2)
        tile_mm2(TILES - 1)


def build_program():
    if 'nc' in _prog_cache:
        return _prog_cache['nc']
    nc = bacc.Bacc()
    dram = declare_io(nc)
    with tile.TileContext(nc) as tc:
        emit(nc, tc, dram)
    nc.finalize()
    _prog_cache['nc'] = nc
    return nc


def _prep_inputs(input1, input2, mempool):
    units = np.concatenate([
        np.asarray(input1, dtype=np.float32).reshape(16, DIM, QPU),
        np.asarray(input2, dtype=np.float32).reshape(16, DIM, QPU),
    ], axis=0)                                      # [32, 512, 1024]
    qh = units.astype(np.float16)
    ql = units - qh.astype(np.float32)
    qx16 = (qh.astype(np.float32) * 16).astype(np.float16)
    qx8 = np.stack([
        (qh.astype(np.float32) * 2.0**-5).astype(E4),
        (ql * 2.0**8).astype(E4),
    ], axis=1)                                      # [32, 2, 512, 1024]

    # pack per (unit, tile, partition): 1KB fp16 | 1KB fp8
    b16 = (qx16.reshape(32, 4, 128, 8, 128).transpose(0, 3, 2, 1, 4)
           .reshape(32, 8, 128, 512).view(np.uint8))          # [u,tt,p,1024B]
    b8 = (qx8.reshape(32, 2, 4, 128, 8, 128).transpose(0, 4, 3, 1, 2, 5)
          .reshape(32, 8, 128, 1024).view(np.uint8))          # [u,tt,p,1024B]
    qall = np.ascontiguousarray(np.concatenate([b16, b8], axis=3)).view(np.float16)

    mpT = np.ascontiguousarray(np.asarray(mempool, dtype=np.float32).T)
    mh = mpT.astype(np.float16).astype(np.float32)
    ml = mpT - mh
    mh16 = (mh * 16).astype(np.float16)
    m8 = np.stack([(ml * 2.0**13).astype(E4), mh.astype(E4)])   # [2, 512, 2048]
    mp16 = np.asarray(mempool, dtype=np.float32).astype(np.float16)

    return [{
        "qall": np.ascontiguousarray(qall[4 * k:4 * (k + 1)]),
        "mh16": mh16, "m8": m8, "mp": mp16,
    } for k in range(NCORES)]


def _assemble(results):
    outs = np.empty((32, DIM, QPU), dtype=np.float32)
    for k in range(NCORES):
        o = results[k]["out"]
        for j in range(UNITS_PER_CORE):
            outs[4 * k + j] = o[QPU * j:QPU * (j + 1), :].T
    return outs[:16].reshape(16, DIM, 32, 32), outs[16:].reshape(16, DIM, 32, 32)


def kernel(input1, input2, mempool):
    nc = build_program()
    in_maps = _prep_inputs(input1, input2, mempool)
    res = run_bass_kernel_spmd(nc, in_maps, core_ids=list(range(NCORES)))
    return _assemble(res.results)


if __name__ == "__main__":
    rng = np.random.default_rng(0)
    i1 = rng.standard_normal((16, DIM, 32, 32)).astype(np.float32)
    i2 = rng.standard_normal((16, DIM, 32, 32)).astype(np.float32)
    mp = rng.uniform(-1 / np.sqrt(DIM), 1 / np.sqrt(DIM), (NITEM, DIM)).astype(np.float32)
    o1, o2 = kernel(i1, i2, mp)
    print("ok", o1.shape, o2.shape, o1.dtype)


# revision 25
# speedup vs baseline: 1.0487x; 1.0487x over previous
"""Trainium2 Bass kernel for nn_Memory (topk_masking).

Algorithm (per query row q of N=32768, item count 2048, K=10):
  logits l = q @ mempool.T
  e = exp(l); S = sum(e)                       (softmax, no max-shift: |l| <= ~3)
  top-10 threshold t10 via 8-way sectioned DVE max8 + small merge
  u = 1 + e/S  (~= exp(e/S) to 5e-6: e/S <= 4e-3)
  g = (e >= t10) * u;  Z = sum(g)
  out = (g @ mempool) / Z                      (masked dense matmul, no gather)

mm1 precision: fp16 main term (16*qh)@(16*mh) + one fp8-e4m3 DoubleRow matmul
carrying both correction terms (qh/32 paired with 2^13*ml, 2^8*ql paired with
mh) — every product lands at 2^8 scale in PSUM; exp folds in the 2^-8.
Logit noise sigma ~ 6.6e-6 -> ~6 selection flips per 32768 rows, rel ~5e-3.
DoubleRow runs at 0.5 cyc/row: mm1 = 12288 PE-cyc/tile vs 24576 for the
3-term fp16 split. mm2 dense fp16 (8192 cyc).

Schedule: mm2 skewed 2 tiles behind mm1 so the topk/mask/transpose chain of
tile t hides under mm1(t+1), mm1(t+2). DMAs spread over three queues to kill
head-of-line serialization: SP = loads, Act-HWDGE = g transposes,
Pool-SWDGE = out stores. Per-tile q data packed into a single 2KB/partition
transfer.

Sharding: data-parallel over queries. 32 units of [512 dim x 1024 queries]
(16 batches x 2 inputs); each of 8 cores takes 4 units = 32 tiles of 128
queries. mempool (4MB) replicated per core. Host does layout marshalling only.
"""
import sys
sys.path.insert(0, '/opt/trn_rl_repo')

import numpy as np
import ml_dtypes
import concourse.bacc as bacc
import concourse.mybir as mybir
import concourse.tile as tile
from concourse.bass_utils import run_bass_kernel_spmd

F32 = mybir.dt.float32
F16 = mybir.dt.float16
F8E4 = mybir.dt.float8e4
E4 = ml_dtypes.float8_e4m3

DIM = 512
NITEM = 2048
NCORES = 8
UNITS_PER_CORE = 4
QPU = 1024
TILES = UNITS_PER_CORE * QPU // 128
NEG = -1e30
EXP = mybir.ActivationFunctionType.Exp
COPY = mybir.ActivationFunctionType.Copy
DR = mybir.MatmulPerfMode.DoubleRow

_prog_cache = {}


def declare_io(nc):
    return {
        # per-tile packed q payload: [u, tile, partition, 1024 f16] where the
        # 2KB per partition = 1KB fp16 main operand + 1KB fp8 DR pair operand
        "qall": nc.declare_dram_parameter("qall", [UNITS_PER_CORE, 8, 128, 1024],
                                          F16, isOutput=False),
        "mh16": nc.declare_dram_parameter("mh16", [DIM, NITEM], F16, isOutput=False),
        "m8": nc.declare_dram_parameter("m8", [2, DIM, NITEM], F8E4, isOutput=False),
        "mp": nc.declare_dram_parameter("mp", [NITEM, DIM], F16, isOutput=False),
        "out": nc.declare_dram_parameter("out", [UNITS_PER_CORE * QPU, DIM], F16,
                                         isOutput=True),
    }


def emit(nc, tc, dram):
    with (
        tc.tile_pool(name="const", bufs=1) as cpool,
        tc.tile_pool(name="qin", bufs=3) as qpool,
        tc.tile_pool(name="work", bufs=2) as wpool,
        tc.tile_pool(name="carry4", bufs=4) as kpool,
        tc.tile_pool(name="outp", bufs=3) as opool,
        tc.tile_pool(name="ps_l", bufs=2, space="PSUM") as ps_l,
        tc.tile_pool(name="ps_o", bufs=2, space="PSUM") as ps_o,
    ):
        # consts as quarter-tiles: dependency tracking is tile-granular, and
        # HWDGE/DMA engines are globally shared, so emission order ~ global
        # transfer order. Strict first-needed-first lets mm1(0) c2-0 start
        # after just qall0 + mh_q0 + m8_q0 (~4.5 KB/partition).
        mh_q = [cpool.tile([128, 4, 512], F16, name=f"mh_q{i}") for i in range(4)]
        m8_q = [cpool.tile([128, 2, 4, 512], F8E4, name=f"m8_q{i}") for i in range(4)]
        mp_sb = cpool.tile([128, 16, DIM], F16)

        qall0_sb = qpool.tile([128, 1024], F16, tag="qall", name="qall_sb")
        nc.sync.dma_start(qall0_sb[:], dram["qall"][0, 0])
        qeng = [nc.sync, nc.scalar]
        for q in range(4):
            qs = slice(512 * q, 512 * (q + 1))
            qeng[q % 2].dma_start(mh_q[q][:],
                                  dram["mh16"][:, qs].rearrange("(kc p) n -> p kc n", p=128))
            qeng[(q + 1) % 2].dma_start(m8_q[q][:],
                                        dram["m8"][:, :, qs].rearrange("two (kc p) n -> p two kc n", p=128))
        # gate mp behind the last const quarter (Pool SEQ is in-order): its
        # 5.8us transfer otherwise grabs the DMA engines ahead of the
        # startup-critical quarters
        gate = cpool.tile([128, 1], F16, name="gate")
        nc.gpsimd.tensor_scalar_mul(gate[:], mh_q[3][:, 0, 0:1], 1.0)
        nc.gpsimd.tensor_scalar_mul(gate[:], m8_q[3][:, 0, 0, 0:2].bitcast(F16), 1.0)
        for hc in range(2):
            rs = slice(1024 * hc, 1024 * (hc + 1))
            nc.gpsimd.dma_start(mp_sb[:, 8 * hc:8 * (hc + 1), :],
                                dram["mp"][rs].rearrange("(ic p) d -> p ic d", p=128))

        carry = {}

        def tile_mm1(t):
            u, tt = divmod(t, QPU // 128)
            if t == 0:
                qall_sb = qall0_sb
            else:
                qall_sb = qpool.tile([128, 1024], F16, tag="qall", name="qall_sb")
                nc.sync.dma_start(qall_sb[:], dram["qall"][u, tt])
            q16_sb = qall_sb[:, 0:512].rearrange("p (kc f) -> p kc f", kc=4)
            q8_sb = (qall_sb[:, 512:1024].bitcast(F8E4)
                     .rearrange("p (two kc f) -> p two kc f", two=2, kc=4))

            e_sb = wpool.tile([128, NITEM], F32, tag="e", name="e_sb")
            S_p = wpool.tile([128, 2], F32, tag="Sp", name="S_p")
            cand = wpool.tile([128, 64], F32, tag="cand", name="cand")
            for h in range(2):
                l_ps = ps_l.tile([128, 1024], F32, tag="l", name="l_ps")
                for c2 in range(2):
                    qi = 2 * h + c2
                    dst = l_ps[:, 512 * c2:512 * (c2 + 1)]
                    for kc in range(4):
                        nc.tensor.matmul(dst, q16_sb[:, kc, :],
                                         mh_q[qi][:, kc, :],
                                         start=(kc == 0), stop=False)
                    for kc in range(4):
                        nc.tensor.matmul(dst, q8_sb[:, :, kc, :],
                                         m8_q[qi][:, :, kc, :],
                                         start=False, stop=(kc == 3), perf_mode=DR)
                nc.scalar.activation(e_sb[:, 1024 * h:1024 * (h + 1)], l_ps[:],
                                     EXP, scale=2.0**-8, accum_out=S_p[:, h:h + 1])
                # sectioned top-8 for this half overlaps the other half's matmul
                for s in range(4 * h, 4 * h + 4):
                    nc.vector.max(out=cand[:, 8 * s:8 * (s + 1)],
                                  in_=e_sb[:, 256 * s:256 * (s + 1)])

            S_sb = wpool.tile([128, 1], F32, tag="S", name="S_sb")
            nc.vector.tensor_add(S_sb[:], S_p[:, 0:1], S_p[:, 1:2])
            Sinv = wpool.tile([128, 1], F32, tag="Sinv", name="Sinv")
            nc.vector.reciprocal(Sinv[:], S_sb[:])

            top8 = wpool.tile([128, 8], F32, tag="top8", name="top8")
            candm = wpool.tile([128, 64], F32, tag="candm", name="candm")
            next8 = wpool.tile([128, 8], F32, tag="next8", name="next8")
            nc.vector.max(out=top8[:], in_=cand[:])
            nc.vector.match_replace(out=candm[:], in_to_replace=top8[:],
                                    in_values=cand[:], imm_value=NEG)
            nc.vector.max(out=next8[:], in_=candm[:])

            # g / transpose in item-halves: halves the drain latency of the
            # stt -> transpose -> mm2 chain (matters at the pipeline tail)
            u_sb = wpool.tile([128, NITEM], F16, tag="u", name="u_sb")
            g_sb = wpool.tile([128, NITEM], F16, tag="g", name="g_sb")
            Z_p = wpool.tile([128, 2], F32, tag="Zp", name="Z_p")
            gtA = kpool.tile([128, 8, 128], F16, tag="gtA", name="gtA")
            gtB = kpool.tile([128, 8, 128], F16, tag="gtB", name="gtB")
            for h, (gt_h, eng) in enumerate([(gtA, nc.scalar), (gtB, nc.scalar)]):
                cs = slice(1024 * h, 1024 * (h + 1))
                nc.scalar.activation(u_sb[:, cs], e_sb[:, cs], COPY,
                                     scale=Sinv[:], bias=1.0)
                nc.vector.scalar_tensor_tensor(
                    out=g_sb[:, cs], in0=e_sb[:, cs], scalar=next8[:, 1:2],
                    in1=u_sb[:, cs],
                    op0=mybir.AluOpType.is_ge, op1=mybir.AluOpType.mult,
                    accum_out=Z_p[:, h:h + 1])
                eng.dma_start_transpose(gt_h[:], g_sb[:, cs])
            Z_sb = wpool.tile([128, 1], F32, tag="Z", name="Z_sb")
            nc.vector.tensor_add(Z_sb[:], Z_p[:, 0:1], Z_p[:, 1:2])
            Zinv = kpool.tile([128, 1], F32, tag="Zinv", name="Zinv")
            nc.vector.reciprocal(Zinv[:], Z_sb[:])
            carry[t] = (gtA, gtB, Zinv)

        def tile_mm2(t):
            gtA, gtB, Zinv = carry.pop(t)
            o_ps = ps_o.tile([128, DIM], F32, tag="o", name="o_ps")
            for ic in range(16):
                gt_h = gtA if ic < 8 else gtB
                nc.tensor.matmul(o_ps[:], gt_h[:, ic % 8, :], mp_sb[:, ic, :],
                                 start=(ic == 0), stop=(ic == 15))
            o_sb = opool.tile([128, DIM], F16, tag="osb", name="o_sb")
            nc.scalar.activation(o_sb[:], o_ps[:], COPY, scale=Zinv[:])
            nc.gpsimd.dma_start(dram["out"][128 * t:128 * (t + 1), :], o_sb[:])

        # software pipeline, skew 3: the topk/mask/transpose chain of tile t
        # completes while PE runs mm1(t+1..t+3)
        SKEW = 3
        for t in range(TILES):
            tile_mm1(t)
            if t >= SKEW:
                tile_mm2(t - SKEW)
        for t in range(TILES - SKEW, TILES):
            tile_mm2(t)


def build_program():
    if 'nc' in _prog_cache:
        return _prog_cache['nc']
    nc = bacc.Bacc()
    dram = declare_io(nc)
    with tile.TileContext(nc) as tc:
        emit(nc, tc, dram)
    nc.finalize()
    _prog_cache['nc'] = nc
    return nc


def _prep_inputs(input1, input2, mempool):
    units = np.concatenate([
        np.asarray(input1, dtype=np.float32).reshape(16, DIM, QPU),
        np.asarray(input2, dtype=np.float32).reshape(16, DIM, QPU),
    ], axis=0)                                      # [32, 512, 1024]
    qh = units.astype(np.float16)
    ql = units - qh.astype(np.float32)
    qx16 = (qh.astype(np.float32) * 16).astype(np.float16)
    qx8 = np.stack([
        (qh.astype(np.float32) * 2.0**-5).astype(E4),
        (ql * 2.0**8).astype(E4),
    ], axis=1)                                      # [32, 2, 512, 1024]

    # pack per (unit, tile, partition): 1KB fp16 | 1KB fp8
    b16 = (qx16.reshape(32, 4, 128, 8, 128).transpose(0, 3, 2, 1, 4)
           .reshape(32, 8, 128, 512).view(np.uint8))          # [u,tt,p,1024B]
    b8 = (qx8.reshape(32, 2, 4, 128, 8, 128).transpose(0, 4, 3, 1, 2, 5)
          .reshape(32, 8, 128, 1024).view(np.uint8))          # [u,tt,p,1024B]
    qall = np.ascontiguousarray(np.concatenate([b16, b8], axis=3)).view(np.float16)

    mpT = np.ascontiguousarray(np.asarray(mempool, dtype=np.float32).T)
    mh = mpT.astype(np.float16).astype(np.float32)
    ml = mpT - mh
    mh16 = (mh * 16).astype(np.float16)
    m8 = np.stack([(ml * 2.0**13).astype(E4), mh.astype(E4)])   # [2, 512, 2048]
    mp16 = np.asarray(mempool, dtype=np.float32).astype(np.float16)

    return [{
        "qall": np.ascontiguousarray(qall[4 * k:4 * (k + 1)]),
        "mh16": mh16, "m8": m8, "mp": mp16,
    } for k in range(NCORES)]


def _assemble(results):
    outs = np.empty((32, DIM, QPU), dtype=np.float32)
    for k in range(NCORES):
        o = results[k]["out"]
        for j in range(UNITS_PER_CORE):
            outs[4 * k + j] = o[QPU * j:QPU * (j + 1), :].T.astype(np.float32)
    return outs[:16].reshape(16, DIM, 32, 32), outs[16:].reshape(16, DIM, 32, 32)


def kernel(input1, input2, mempool):
    nc = build_program()
    in_maps = _prep_inputs(input1, input2, mempool)
    res = run_bass_kernel_spmd(nc, in_maps, core_ids=list(range(NCORES)))
    return _assemble(res.results)


if __name__ == "__main__":
    rng = np.random.default_rng(0)
    i1 = rng.standard_normal((16, DIM, 32, 32)).astype(np.float32)
    i2 = rng.standard_normal((16, DIM, 32, 32)).astype(np.float32)
    mp = rng.uniform(-1 / np.sqrt(DIM), 1 / np.sqrt(DIM), (NITEM, DIM)).astype(np.float32)
    o1, o2 = kernel(i1, i2, mp)
    print("ok", o1.shape, o2.shape, o1.dtype)


# revision 27
# speedup vs baseline: 1.0958x; 1.0449x over previous
"""Trainium2 Bass kernel for nn_Memory (topk_masking).

Algorithm (per query row q of N=32768, item count 2048, K=10):
  logits l = q @ mempool.T
  e = exp(l); S = sum(e)                       (softmax, no max-shift: |l| <= ~3)
  top-10 threshold t10 via 8-way sectioned DVE max8 + small merge
  u = 1 + e/S  (~= exp(e/S) to 5e-6: e/S <= 4e-3)
  g = (e >= t10) * u;  Z = sum(g)
  out = (g @ mempool) / Z                      (masked dense matmul, no gather)

mm1 precision: fp16 main term (16*qh)@(16*mh) + one fp8-e4m3 DoubleRow matmul
carrying both correction terms (qh/32 paired with 2^13*ml, 2^8*ql paired with
mh) — every product lands at 2^8 scale in PSUM; exp folds in the 2^-8.
Logit noise sigma ~ 6.6e-6 -> ~6 selection flips per 32768 rows, rel ~5e-3.
DoubleRow runs at 0.5 cyc/row: mm1 = 12288 PE-cyc/tile vs 24576 for the
3-term fp16 split. mm2 dense fp16 (8192 cyc).

Schedule: mm2 skewed 2 tiles behind mm1 so the topk/mask/transpose chain of
tile t hides under mm1(t+1), mm1(t+2). DMAs spread over three queues to kill
head-of-line serialization: SP = loads, Act-HWDGE = g transposes,
Pool-SWDGE = out stores. Per-tile q data packed into a single 2KB/partition
transfer.

Sharding: data-parallel over queries. 32 units of [512 dim x 1024 queries]
(16 batches x 2 inputs); each of 8 cores takes 4 units = 32 tiles of 128
queries. mempool (4MB) replicated per core. Host does layout marshalling only.
"""
import sys
sys.path.insert(0, '/opt/trn_rl_repo')

import numpy as np
import ml_dtypes
import concourse.bacc as bacc
import concourse.mybir as mybir
import concourse.tile as tile
from concourse.bass_utils import run_bass_kernel_spmd

F32 = mybir.dt.float32
F16 = mybir.dt.float16
F8E4 = mybir.dt.float8e4
E4 = ml_dtypes.float8_e4m3

DIM = 512
NITEM = 2048
NCORES = 8
UNITS_PER_CORE = 4
QPU = 1024
TILES = UNITS_PER_CORE * QPU // 128
NEG = -1e30
EXP = mybir.ActivationFunctionType.Exp
COPY = mybir.ActivationFunctionType.Copy
DR = mybir.MatmulPerfMode.DoubleRow

_prog_cache = {}


def declare_io(nc):
    return {
        # per-tile packed q payload: [u, tile, partition, 1024 f16] where the
        # 2KB per partition = 1KB fp16 main operand + 1KB fp8 DR pair operand
        "qall": nc.declare_dram_parameter("qall", [UNITS_PER_CORE, 8, 128, 1024],
                                          F16, isOutput=False),
        "mh16": nc.declare_dram_parameter("mh16", [DIM, NITEM], F16, isOutput=False),
        "m8": nc.declare_dram_parameter("m8", [2, DIM, NITEM], F8E4, isOutput=False),
        "mp": nc.declare_dram_parameter("mp", [NITEM, DIM], F16, isOutput=False),
        "out": nc.declare_dram_parameter("out", [UNITS_PER_CORE * QPU, DIM], F16,
                                         isOutput=True),
    }


def emit(nc, tc, dram):
    with (
        tc.tile_pool(name="const", bufs=1) as cpool,
        tc.tile_pool(name="qin", bufs=3) as qpool,
        tc.tile_pool(name="work", bufs=2) as wpool,
        tc.tile_pool(name="carry4", bufs=4) as kpool,
        tc.tile_pool(name="outp", bufs=3) as opool,
        tc.tile_pool(name="ps_l", bufs=3, space="PSUM") as ps_l,
        tc.tile_pool(name="ps_o", bufs=2, space="PSUM") as ps_o,
    ):
        # consts as quarter-tiles: dependency tracking is tile-granular, and
        # HWDGE/DMA engines are globally shared, so emission order ~ global
        # transfer order. Strict first-needed-first lets mm1(0) c2-0 start
        # after just qall0 + mh_q0 + m8_q0 (~4.5 KB/partition).
        mh_q = [cpool.tile([128, 4, 512], F16, name=f"mh_q{i}") for i in range(4)]
        m8_q = [cpool.tile([128, 2, 4, 512], F8E4, name=f"m8_q{i}") for i in range(4)]
        mp_sb = cpool.tile([128, 16, DIM], F16)

        qall0_sb = qpool.tile([128, 1024], F16, tag="qall", name="qall_sb")
        nc.sync.dma_start(qall0_sb[:], dram["qall"][0, 0])
        qeng = [nc.sync, nc.scalar]
        for q in range(4):
            qs = slice(512 * q, 512 * (q + 1))
            qeng[q % 2].dma_start(mh_q[q][:],
                                  dram["mh16"][:, qs].rearrange("(kc p) n -> p kc n", p=128))
            qeng[(q + 1) % 2].dma_start(m8_q[q][:],
                                        dram["m8"][:, :, qs].rearrange("two (kc p) n -> p two kc n", p=128))
        # gate mp behind the last const quarter (Pool SEQ is in-order): its
        # 5.8us transfer otherwise grabs the DMA engines ahead of the
        # startup-critical quarters
        gate = cpool.tile([128, 1], F16, name="gate")
        nc.gpsimd.tensor_scalar_mul(gate[:], mh_q[3][:, 0, 0:1], 1.0)
        nc.gpsimd.tensor_scalar_mul(gate[:], m8_q[3][:, 0, 0, 0:2].bitcast(F16), 1.0)
        for hc in range(2):
            rs = slice(1024 * hc, 1024 * (hc + 1))
            nc.gpsimd.dma_start(mp_sb[:, 8 * hc:8 * (hc + 1), :],
                                dram["mp"][rs].rearrange("(ic p) d -> p ic d", p=128))

        carry = {}

        def tile_mm1(t):
            u, tt = divmod(t, QPU // 128)
            if t == 0:
                qall_sb = qall0_sb
            else:
                qall_sb = qpool.tile([128, 1024], F16, tag="qall", name="qall_sb")
                nc.sync.dma_start(qall_sb[:], dram["qall"][u, tt])
            q16_sb = qall_sb[:, 0:512].rearrange("p (kc f) -> p kc f", kc=4)
            q8_sb = (qall_sb[:, 512:1024].bitcast(F8E4)
                     .rearrange("p (two kc f) -> p two kc f", two=2, kc=4))

            e_sb = wpool.tile([128, NITEM], F32, tag="e", name="e_sb")
            S_p = wpool.tile([128, 2], F32, tag="Sp", name="S_p")
            cand = wpool.tile([128, 64], F32, tag="cand", name="cand")
            for h in range(2):
                l_ps = ps_l.tile([128, 1024], F32, tag="l", name="l_ps")
                for c2 in range(2):
                    qi = 2 * h + c2
                    dst = l_ps[:, 512 * c2:512 * (c2 + 1)]
                    for kc in range(4):
                        nc.tensor.matmul(dst, q16_sb[:, kc, :],
                                         mh_q[qi][:, kc, :],
                                         start=(kc == 0), stop=False)
                    for kc in range(4):
                        nc.tensor.matmul(dst, q8_sb[:, :, kc, :],
                                         m8_q[qi][:, :, kc, :],
                                         start=False, stop=(kc == 3), perf_mode=DR)
                nc.scalar.activation(e_sb[:, 1024 * h:1024 * (h + 1)], l_ps[:],
                                     EXP, scale=2.0**-8, accum_out=S_p[:, h:h + 1])
                # sectioned top-8 for this half overlaps the other half's matmul
                for s in range(4 * h, 4 * h + 4):
                    nc.vector.max(out=cand[:, 8 * s:8 * (s + 1)],
                                  in_=e_sb[:, 256 * s:256 * (s + 1)])

            S_sb = wpool.tile([128, 1], F32, tag="S", name="S_sb")
            nc.vector.tensor_add(S_sb[:], S_p[:, 0:1], S_p[:, 1:2])
            Sinv = wpool.tile([128, 1], F32, tag="Sinv", name="Sinv")
            nc.vector.reciprocal(Sinv[:], S_sb[:])

            top8 = wpool.tile([128, 8], F32, tag="top8", name="top8")
            candm = wpool.tile([128, 64], F32, tag="candm", name="candm")
            next8 = wpool.tile([128, 8], F32, tag="next8", name="next8")
            nc.vector.max(out=top8[:], in_=cand[:])
            nc.vector.match_replace(out=candm[:], in_to_replace=top8[:],
                                    in_values=cand[:], imm_value=NEG)
            nc.vector.max(out=next8[:], in_=candm[:])

            # g / transpose in item-halves: halves the drain latency of the
            # stt -> transpose -> mm2 chain (matters at the pipeline tail)
            u_sb = wpool.tile([128, NITEM], F16, tag="u", name="u_sb")
            g_sb = wpool.tile([128, NITEM], F16, tag="g", name="g_sb")
            Z_p = wpool.tile([128, 2], F32, tag="Zp", name="Z_p")
            gtA = kpool.tile([128, 8, 128], F16, tag="gtA", name="gtA")
            gtB = kpool.tile([128, 8, 128], F16, tag="gtB", name="gtB")
            for h, (gt_h, eng) in enumerate([(gtA, nc.scalar), (gtB, nc.scalar)]):
                cs = slice(1024 * h, 1024 * (h + 1))
                nc.scalar.activation(u_sb[:, cs], e_sb[:, cs], COPY,
                                     scale=Sinv[:], bias=1.0)
                nc.vector.scalar_tensor_tensor(
                    out=g_sb[:, cs], in0=e_sb[:, cs], scalar=next8[:, 1:2],
                    in1=u_sb[:, cs],
                    op0=mybir.AluOpType.is_ge, op1=mybir.AluOpType.mult,
                    accum_out=Z_p[:, h:h + 1])
                eng.dma_start_transpose(gt_h[:], g_sb[:, cs])
            Z_sb = wpool.tile([128, 1], F32, tag="Z", name="Z_sb")
            nc.vector.tensor_add(Z_sb[:], Z_p[:, 0:1], Z_p[:, 1:2])
            Zinv = kpool.tile([128, 1], F32, tag="Zinv", name="Zinv")
            nc.vector.reciprocal(Zinv[:], Z_sb[:])
            carry[t] = (gtA, gtB, Zinv)

        def tile_mm2(t):
            gtA, gtB, Zinv = carry.pop(t)
            o_ps = ps_o.tile([128, DIM], F32, tag="o", name="o_ps")
            for ic in range(16):
                gt_h = gtA if ic < 8 else gtB
                nc.tensor.matmul(o_ps[:], gt_h[:, ic % 8, :], mp_sb[:, ic, :],
                                 start=(ic == 0), stop=(ic == 15))
            o_sb = opool.tile([128, DIM], F16, tag="osb", name="o_sb")
            nc.scalar.activation(o_sb[:], o_ps[:], COPY, scale=Zinv[:])
            nc.gpsimd.dma_start(dram["out"][128 * t:128 * (t + 1), :], o_sb[:])

        # software pipeline, skew 3: the topk/mask/transpose chain of tile t
        # completes while PE runs mm1(t+1..t+3)
        SKEW = 3
        for t in range(TILES):
            tile_mm1(t)
            if t >= SKEW:
                tile_mm2(t - SKEW)
        for t in range(TILES - SKEW, TILES):
            tile_mm2(t)


def build_program():
    if 'nc' in _prog_cache:
        return _prog_cache['nc']
    nc = bacc.Bacc()
    dram = declare_io(nc)
    with tile.TileContext(nc) as tc:
        emit(nc, tc, dram)
    nc.finalize()
    _prog_cache['nc'] = nc
    return nc


def _prep_inputs(input1, input2, mempool):
    units = np.concatenate([
        np.asarray(input1, dtype=np.float32).reshape(16, DIM, QPU),
        np.asarray(input2, dtype=np.float32).reshape(16, DIM, QPU),
    ], axis=0)                                      # [32, 512, 1024]
    qh = units.astype(np.float16)
    ql = units - qh.astype(np.float32)
    qx16 = (qh.astype(np.float32) * 16).astype(np.float16)
    qx8 = np.stack([
        (qh.astype(np.float32) * 2.0**-5).astype(E4),
        (ql * 2.0**8).astype(E4),
    ], axis=1)                                      # [32, 2, 512, 1024]

    # pack per (unit, tile, partition): 1KB fp16 | 1KB fp8
    b16 = (qx16.reshape(32, 4, 128, 8, 128).transpose(0, 3, 2, 1, 4)
           .reshape(32, 8, 128, 512).view(np.uint8))          # [u,tt,p,1024B]
    b8 = (qx8.reshape(32, 2, 4, 128, 8, 128).transpose(0, 4, 3, 1, 2, 5)
          .reshape(32, 8, 128, 1024).view(np.uint8))          # [u,tt,p,1024B]
    qall = np.ascontiguousarray(np.concatenate([b16, b8], axis=3)).view(np.float16)

    mpT = np.ascontiguousarray(np.asarray(mempool, dtype=np.float32).T)
    mh = mpT.astype(np.float16).astype(np.float32)
    ml = mpT - mh
    mh16 = (mh * 16).astype(np.float16)
    m8 = np.stack([(ml * 2.0**13).astype(E4), mh.astype(E4)])   # [2, 512, 2048]
    mp16 = np.asarray(mempool, dtype=np.float32).astype(np.float16)

    return [{
        "qall": np.ascontiguousarray(qall[4 * k:4 * (k + 1)]),
        "mh16": mh16, "m8": m8, "mp": mp16,
    } for k in range(NCORES)]


def _assemble(results):
    outs = np.empty((32, DIM, QPU), dtype=np.float32)
    for k in range(NCORES):
        o = results[k]["out"]
        for j in range(UNITS_PER_CORE):
            outs[4 * k + j] = o[QPU * j:QPU * (j + 1), :].T.astype(np.float32)
    return outs[:16].reshape(16, DIM, 32, 32), outs[16:].reshape(16, DIM, 32, 32)


def kernel(input1, input2, mempool):
    nc = build_program()
    in_maps = _prep_inputs(input1, input2, mempool)
    res = run_bass_kernel_spmd(nc, in_maps, core_ids=list(range(NCORES)))
    return _assemble(res.results)


if __name__ == "__main__":
    rng = np.random.default_rng(0)
    i1 = rng.standard_normal((16, DIM, 32, 32)).astype(np.float32)
    i2 = rng.standard_normal((16, DIM, 32, 32)).astype(np.float32)
    mp = rng.uniform(-1 / np.sqrt(DIM), 1 / np.sqrt(DIM), (NITEM, DIM)).astype(np.float32)
    o1, o2 = kernel(i1, i2, mp)
    print("ok", o1.shape, o2.shape, o1.dtype)
